# revision 21
# baseline (speedup 1.0000x reference)
"""Single-head causal attention on 8 Trainium2 NeuronCores.

Sharding: core = 2*b + c handles batch b (of 4) and query rows {2j+c}
(1024 rows) — balanced causal work per core, no collectives (inputs are
replicated host-side).

Algebra: scores = Q@K.T = x @ (Wk.T@Wq) @ x.T and (attn@V)@Wo.T =
attn @ (x@(Wo@Wv).T), so with host-precomputed G = Wk.T@Wq and
Wvo = Wo@Wv (exact fp32) the device only runs:
  QT[i,q]   = G @ xT[:, :1024]          (1 projection)
  VWo[l,o]  = x-chunks.T @ WvoT          (1 projection)
  S.T[l,q]  = xT-chunks.T @ QT           -> expT = exp(S.T/32) * causal_mask
  out[q,o]  = (expT.T @ VWo) / (expT.T @ 1)
All matmuls bf16 with fp32 PSUM accumulate; x columns are permuted per
core so its q rows are xT cols 0..1023 and the causal structure is the
same compile-time pattern on every core.

Optimizations over the first working version (1024 us/core -> ~128 us/core
steady-state marginal span, measured by interleaved repeat-scaling):
 - pair-wise VWo exchange: the x@Wvo.T projection over all 2048 rows was
   computed redundantly by both cores of a batch.  Now each core computes
   only its own-parity half (chunks 0..7), publishes it to DRAM, AllGathers
   within the pair, and pulls the partner half back with an indirect DMA
   driven by a per-core row-index tensor (the rank asymmetry lives in data,
   keeping the SPMD program uniform).  Cuts PE streaming work by 19%.
 - causal masks generated on-device once (affine_select iota + a per-core
   [128,128] diagonal-correction tile) instead of 2 MiB of mask DMAs.
 - batched input DMAs (7 loads vs 44), x double-buffered across repeats,
   weights loaded once; load order (wvo, x-q-half first) lets A2 start
   while the rest streams in and hides the exchange behind A1 + scores.
 - bf16 output store (host upcasts; output rounding adds ~0.001 rel err).
"""

import os
import numpy as np
import ml_dtypes

import concourse.bass as bass
import concourse.bacc as bacc
import concourse.mybir as mybir
import concourse.tile as tile
from concourse.bass_utils import run_bass_kernel_spmd

BF16 = ml_dtypes.bfloat16
B, S, D = 4, 2048, 1024
NC = 8          # i-chunks of 128 (contraction dim)
NL = 16         # l-chunks of 128
# score chunk-pairs (j, j+8) computed per q-block; "mixed" pairs get the
# causal mask; off = 128*(j-4*qb) = first un-masked q column in the block
PAIRS = {0: [0, 1, 2, 3], 1: [0, 1, 2, 3, 4, 5, 6, 7]}
MIXEDP = {0: [0, 1, 2, 3], 1: [4, 5, 6, 7]}

LAST_EXEC_TIME_NS = None
LAST_RESULTS = None
_CACHE = {}


def _build(with_biases: bool, repeat: int = 1, load_once: bool = False,
           exchange: bool = True):
    f32, bf16 = mybir.dt.float32, mybir.dt.bfloat16
    nc = bacc.Bacc("TRN2", target_bir_lowering=False, debug=False, num_devices=8)

    xT = nc.dram_tensor("xT", [128, NC, S], bf16, kind="ExternalInput")
    wg = nc.dram_tensor("wg", [128, NC, D], bf16, kind="ExternalInput")   # G.T layout
    wvo = nc.dram_tensor("wvo", [128, NC, D], bf16, kind="ExternalInput")  # Wvo.T layout
    dmk = nc.dram_tensor("dmk", [128, 128], bf16, kind="ExternalInput")   # diag fix
    if with_biases:
        vqd = nc.dram_tensor("vq", [128, NC, 1], bf16, kind="ExternalInput")  # Wk.T@bq
        bvod = nc.dram_tensor("bvo", [1, D], bf16, kind="ExternalInput")      # Wo@bv+bo
    out = nc.dram_tensor("out", [128, NC, 2, 512], bf16, kind="ExternalOutput")
    if exchange:
        # pair-wise VWo exchange: each core computes its own-parity half,
        # AllGathers the halves through DRAM, and gathers the partner half
        # back with a per-core row-index tensor (rank-asymmetry is data)
        vidx = nc.dram_tensor("vidx", [128, 1], mybir.dt.int32, kind="ExternalInput")
        vh_d = nc.dram_tensor("vh", [128, NC * D], bf16, kind="Internal")
        vg_d = nc.dram_tensor("vg", [256, NC * D], bf16, kind="Internal")

    with tile.TileContext(nc) as tc:
      with (
          tc.tile_pool(name="big", bufs=1) as big,
          tc.tile_pool(name="cst", bufs=1) as cst,
          tc.tile_pool(name="psum", bufs=1, space=bass.MemorySpace.PSUM) as psp,
      ):
        qt_sb = big.tile([128, NC, 2, 512], bf16)    # G@xTq [i_loc, ic, qh, q]
        vwo_sb = big.tile([128, NL, 2, 512], bf16)   # x@Wvo.T [l_loc, lt, oh, o]
        wg_sb = big.tile([128, NC, D], bf16)
        wvo_w = big.tile([128, NC, D], bf16)
        dm_sb = cst.tile([128, 128], bf16)

        def load_x(x_sb, first):
            # ordered so the first consumer (A2: wvo @ x[:, :1024]) starts
            # early; weights load once, interleaved with the first x load
            if first:
                nc.sync.dma_start(wvo_w[:, 0:4, :], wvo.ap()[:, 0:4, :])
            nc.sync.dma_start(x_sb[:, 0:4, 0:1024], xT.ap()[:, 0:4, 0:1024])
            if first:
                nc.sync.dma_start(wvo_w[:, 4:8, :], wvo.ap()[:, 4:8, :])
            nc.sync.dma_start(x_sb[:, 4:8, 0:1024], xT.ap()[:, 4:8, 0:1024])
            if first:
                nc.sync.dma_start(wg_sb[:], wg.ap())
            nc.sync.dma_start(x_sb[:, 0:4, 1024:2048], xT.ap()[:, 0:4, 1024:2048])
            nc.sync.dma_start(x_sb[:, 4:8, 1024:2048], xT.ap()[:, 4:8, 1024:2048])

        ones_col = cst.tile([128, 1], bf16)
        nc.vector.memset(ones_col[:], 1.0)
        nc.sync.dma_start(dm_sb[:], dmk.ap())
        if exchange:
            idx_sb = cst.tile([128, 1], mybir.dt.int32)
            nc.sync.dma_start(idx_sb[:], vidx.ap())

        # causal mask tiles, one per distinct q-offset (built once):
        # keep et[:, h, off+s] iff s >= i; half 1's exact diagonal
        # (s == i) is parity-dependent -> fold the per-core dm tile in
        mk_sb = {}
        for off in (0, 128, 256, 384):
            m = cst.tile([128, 2, 512], bf16, name=f"mk{off}")
            nc.vector.memset(m[:, :, off:512], 1.0)
            nc.gpsimd.affine_select(
                m[:, :, off:512], m[:, :, off:512],
                pattern=[[0, 2], [1, 512 - off]],
                compare_op=mybir.AluOpType.is_ge,
                fill=0.0, base=0, channel_multiplier=-1)
            nc.vector.tensor_tensor(m[:, 1, off:off + 128], m[:, 1, off:off + 128],
                                    dm_sb[:], mybir.AluOpType.mult)
            mk_sb[off] = m
        if with_biases:
            ones_row = cst.tile([1, 512], bf16)
            nc.vector.memset(ones_row[:], 1.0)
            vq_sb = cst.tile([128, NC, 1], bf16)
            bvo_sb = cst.tile([1, D], bf16)
            vxl_sb = cst.tile([1, S], bf16)

        if with_biases:
            nc.sync.dma_start(vq_sb[:], vqd.ap())
            nc.sync.dma_start(bvo_sb[:], bvod.ap())
        if load_once:
            x_sb = big.tile([128, NC, S], bf16, tag="x", bufs=1)
            load_x(x_sb, first=True)

        for _rep in range(repeat):
            if not load_once:
                x_sb = big.tile([128, NC, S], bf16, tag="x", bufs=2)
                load_x(x_sb, first=_rep == 0)

            # ---------------- phase A: the two projections ----------------
            # A2: VWo[l, do] = x-chunks.T @ WvoT (+ bvo); with exchange only
            # the own-parity half (chunks 0..7) is computed locally
            for lt in range(8 if exchange else NL):
                pa0 = psp.tile([128, 512], f32, tag="paw", bufs=3 if not with_biases else 2, name="paA2")
                pa1 = psp.tile([128, 512], f32, tag="paw", bufs=3 if not with_biases else 2, name="paA2b")
                for ic in range(NC):
                    lw = x_sb[:, ic, lt * 128:(lt + 1) * 128]
                    st = ic == 0
                    sp = ic == NC - 1 and not with_biases
                    nc.tensor.matmul(pa0[:], lw, wvo_w[:, ic, 0:512], start=st, stop=sp)
                    nc.tensor.matmul(pa1[:], lw, wvo_w[:, ic, 512:1024], start=st, stop=sp)
                if with_biases:
                    nc.tensor.matmul(pa0[:], ones_row[0:1, 0:128], bvo_sb[0:1, 0:512],
                                     start=False, stop=True)
                    nc.tensor.matmul(pa1[:], ones_row[0:1, 0:128], bvo_sb[0:1, 512:1024],
                                     start=False, stop=True)
                nc.vector.tensor_copy(vwo_sb[:, lt, 0, :], pa0[:])
                nc.vector.tensor_copy(vwo_sb[:, lt, 1, :], pa1[:])

            if exchange:
                nc.sync.dma_start(vh_d.ap(), vwo_sb[:, 0:8, :, :].opt())
                nc.gpsimd.collective_compute(
                    "AllGather", mybir.AluOpType.bypass,
                    replica_groups=[[0, 1], [2, 3], [4, 5], [6, 7]],
                    ins=[vh_d.ap()], outs=[vg_d.ap()])
                nc.gpsimd.indirect_dma_start(
                    out=vwo_sb[:, 8:16, :, :].opt(), out_offset=None,
                    in_=vg_d.ap(),
                    in_offset=bass.IndirectOffsetOnAxis(ap=idx_sb[:, 0:1], axis=0))

            # A1: QT = G @ xT[:, 0:1024]
            for dc in range(NC):
                pa0 = psp.tile([128, 512], f32, tag="paw", bufs=3 if not with_biases else 2, name="paA1")
                pa1 = psp.tile([128, 512], f32, tag="paw", bufs=3 if not with_biases else 2, name="paA1b")
                for ic in range(NC):
                    lw = wg_sb[:, ic, dc * 128:(dc + 1) * 128]
                    st, sp = ic == 0, ic == NC - 1
                    nc.tensor.matmul(pa0[:], lw, x_sb[:, ic, 0:512], start=st, stop=sp)
                    nc.tensor.matmul(pa1[:], lw, x_sb[:, ic, 512:1024], start=st, stop=sp)
                nc.scalar.copy(qt_sb[:, dc, 0, :], pa0[:])
                nc.scalar.copy(qt_sb[:, dc, 1, :], pa1[:])

            # bias term bq.K[l]: vxl = (Wk.T bq).T @ xT  [1, S]
            if with_biases:
                for lh in range(4):
                    pv = psp.tile([1, 512], f32, tag="pv", bufs=1)
                    for ic in range(NC):
                        nc.tensor.matmul(pv[:], vq_sb[:, ic, 0:1],
                                         x_sb[:, ic, lh * 512:(lh + 1) * 512],
                                         start=(ic == 0), stop=(ic == NC - 1))
                    nc.vector.tensor_copy(vxl_sb[0:1, lh * 512:(lh + 1) * 512], pv[:])

            # ---------------- phase B: attention ----------------
            # all scores first (they don't touch VWo, so they are PE filler
            # while the exchange is in flight), then the attn tiles largest-
            # first so the kernel tail is the 1-pair tile, with the locally
            # computed VWo half (h=0) accumulated before the gathered half
            expt = {}
            for qb in range(2):
                for j in PAIRS[qb]:
                    # chunk pair (cl, cl+8); for mixed pairs only the q-suffix
                    # off..512 survives masking / is read by the attn stage
                    mixed = j in MIXEDP[qb]
                    off = 128 * (j - 4 * qb) if mixed else 0
                    et = big.tile([128, 2, 512], bf16, tag="exp", bufs=13, name="et")
                    for h in range(2):
                        cl = j + 8 * h
                        ps = psp.tile([128, 512], f32, tag="paw", bufs=3 if not with_biases else 2, name="psS")
                        for ic in range(NC):
                            st = ic == 0
                            sp = ic == NC - 1 and not with_biases
                            nc.tensor.matmul(
                                ps[:, off:512], x_sb[:, ic, cl * 128:(cl + 1) * 128],
                                qt_sb[:, ic, qb, off:512], start=st, stop=sp)
                        if with_biases:
                            nc.tensor.matmul(ps[:, off:512],
                                             vxl_sb[0:1, cl * 128:(cl + 1) * 128],
                                             ones_row[0:1, 0:512 - off],
                                             start=False, stop=True)
                        nc.scalar.activation(et[:, h, off:512], ps[:, off:512],
                                             mybir.ActivationFunctionType.Exp,
                                             scale=1.0 / 32.0)
                    if mixed:
                        nc.vector.tensor_tensor(
                            et[:, :, off:512], et[:, :, off:512],
                            mk_sb[off][:, :, off:512], mybir.AluOpType.mult)
                    expt[qb, j] = et
            for t in range(7, -1, -1):
                qb, tl = t // 4, t % 4
                po0 = psp.tile([128, 512], f32, tag="po0", bufs=2, name="po0")
                po1 = psp.tile([128, 512], f32, tag="po1", bufs=2, name="po1")
                pss = psp.tile([128, 1], f32, tag="pss", bufs=1, name="pss")
                npair = t + 1
                for h in range(2):
                    for i in range(npair):
                        lw = expt[qb, i][:, h, tl * 128:(tl + 1) * 128]
                        st, sp = (i == 0 and h == 0), (i == npair - 1 and h == 1)
                        nc.tensor.matmul(po0[:], lw,
                                         vwo_sb[:, i + 8 * h, 0, :], start=st, stop=sp)
                        nc.tensor.matmul(po1[:], lw,
                                         vwo_sb[:, i + 8 * h, 1, :], start=st, stop=sp)
                        nc.tensor.matmul(pss[:], lw, ones_col[:], start=st, stop=sp)
                rec = big.tile([128, 1], f32, tag="rec", bufs=4, name="rec")
                nc.vector.reciprocal(rec[:], pss[:])
                ot = big.tile([128, 2, 512], bf16, tag="outp", bufs=3, name="ot")
                nc.vector.tensor_scalar_mul(ot[:, 0, :], po0[:], rec[:])
                nc.vector.tensor_scalar_mul(ot[:, 1, :], po1[:], rec[:])
                nc.sync.dma_start(out.ap()[:, t, :, :], ot[:])

    nc.compile()
    return nc


def _host_weights(Wq, Wk, Wv, Wo):
    G = (Wk.T.astype(np.float64) @ Wq.astype(np.float64)).astype(np.float32)
    Wvo = (Wo.astype(np.float64) @ Wv.astype(np.float64)).astype(np.float32)

    def wlayout(W):  # lhsT/rhs layout [i_loc, ic, d] = W[d, i] i.e. W.T chunked
        return np.ascontiguousarray(
            W.T.reshape(8, 128, D).transpose(1, 0, 2)).astype(BF16)

    # QT = G @ xT: lhsT[i, d] = G[d, i] -> wlayout(G)
    # VWo = x @ Wvo.T: rhs[i, do] = Wvo[do, i] -> wlayout(Wvo)
    return wlayout(G), wlayout(Wvo)


def _prep_inputs(x, Wq, bq, Wk, bk, Wv, bv, Wo, bo):
    wg_a, wvo_a = _host_weights(Wq, Wk, Wv, Wo)

    eye = np.eye(128, dtype=np.float32)
    dmasks = {0: (1.0 - eye).astype(BF16), 1: np.ones((128, 128), BF16)}

    with_biases = _CACHE.get("with_biases", False)
    if with_biases:
        vq = (Wk.T.astype(np.float64) @ bq.astype(np.float64)).astype(np.float32)
        vq_a = np.ascontiguousarray(vq.reshape(8, 128, 1).transpose(1, 0, 2)).astype(BF16)
        bvo = (Wo.astype(np.float64) @ bv.astype(np.float64) + bo).astype(np.float32)
        bvo_a = bvo.reshape(1, D).astype(BF16)

    in_maps = []
    for core in range(8):
        b, c = core // 2, core % 2
        perm = np.concatenate([np.arange(c, S, 2), np.arange(1 - c, S, 2)])
        xTp = x[b].T[:, perm]                                  # [D, S]
        xa = np.ascontiguousarray(
            xTp.reshape(8, 128, S).transpose(1, 0, 2)).astype(BF16)
        im = {"xT": xa, "wg": wg_a, "wvo": wvo_a, "dmk": dmasks[c],
              "vidx": ((1 - c) * 128 + np.arange(128, dtype=np.int32)
                       ).reshape(128, 1)}
        if with_biases:
            im["vq"] = vq_a
            im["bvo"] = bvo_a
        in_maps.append(im)
    return in_maps


def kernel(x, Wq, bq, Wk, bk, Wv, bv, Wo, bo):
    global LAST_EXEC_TIME_NS, LAST_RESULTS
    args = [np.asarray(a, np.float32) for a in (Wq, bq, Wk, bk, Wv, bv, Wo, bo)]
    Wq, bq, Wk, bk, Wv, bv, Wo, bo = args
    # bk shifts every score of a query row equally -> cancels in softmax.
    with_biases = any(np.any(a) for a in (bq, bv, bo))
    _CACHE["with_biases"] = with_biases
    exchange = not os.environ.get("BASS_NO_EXCHANGE")
    key = ("nc", with_biases, exchange)
    if key not in _CACHE:
        _CACHE[key] = _build(with_biases, exchange=exchange)
    nc = _CACHE[key]

    x = np.asarray(x, dtype=np.float32)
    in_maps = _prep_inputs(x, Wq, bq, Wk, bk, Wv, bv, Wo, bo)

    res = run_bass_kernel_spmd(nc, in_maps, list(range(8)),
                               trace=bool(os.environ.get("BASS_TRACE")))
    LAST_EXEC_TIME_NS = res.exec_time_ns
    LAST_RESULTS = res

    full = np.empty((B, S, D), dtype=np.float32)
    for core in range(8):
        b, c = core // 2, core % 2
        oc = np.asarray(res.results[core]["out"])     # [128, 8, 2, 512] bf16
        full[b, c::2, :] = (
            oc.transpose(1, 0, 2, 3).reshape(1024, D).astype(np.float32))
    return full


# ---------------- numpy emulation of the device program (for testing) ----
def emulate(x, Wq, bq, Wk, bk, Wv, bv, Wo, bo, cast=True):
    def cst(a):
        return a.astype(BF16).astype(np.float32) if cast else a.astype(np.float32)

    G = (Wk.T.astype(np.float64) @ Wq.astype(np.float64)).astype(np.float32)
    Wvo = (Wo.astype(np.float64) @ Wv.astype(np.float64)).astype(np.float32)
    vq = (Wk.T.astype(np.float64) @ bq.astype(np.float64)).astype(np.float32)
    bvo = (Wo.astype(np.float64) @ bv.astype(np.float64) + bo).astype(np.float32)

    full = np.empty((B, S, D), dtype=np.float32)
    i = np.arange(128)[:, None]
    for core in range(8):
        b, c = core // 2, core % 2
        perm = np.concatenate([np.arange(c, S, 2), np.arange(1 - c, S, 2)])
        xT = cst(x[b].T[:, perm])                        # [D, S]
        QT = cst(cst(G) @ xT[:, :1024])                  # [D, 1024]
        VWo = cst(xT.T @ cst(Wvo).T + bvo[None, :])      # [S, D]
        vxl = cst(vq) @ xT                               # [S]
        outc = np.zeros((1024, D), np.float32)
        for qb in range(2):
            et = {}
            for j in PAIRS[qb]:
                mixed = j in MIXEDP[qb]
                off = 128 * (j - 4 * qb) if mixed else 0
                qs = np.arange(qb * 512 + off, qb * 512 + 512)
                e2 = np.zeros((128, 2, 512), np.float32)
                for h in range(2):
                    cl = j + 8 * h
                    sc = xT[:, cl * 128:(cl + 1) * 128].T @ QT[:, qs]
                    sc = sc + vxl[cl * 128:(cl + 1) * 128][:, None]
                    e = cst(np.exp(sc / 32.0))
                    if mixed:
                        keep = (qs[None, :] - i - 128 * j) >= 0
                        e = e * keep
                        if h == 1 and c == 0:
                            e[:, 0:128] = e[:, 0:128] * (
                                1.0 - np.eye(128, dtype=np.float32))
                    e2[:, h, off:512] = e
                et[j] = e2
            for tl in range(4):
                t = 4 * qb + tl
                num = np.zeros((128, D), np.float32)
                den = np.zeros((128, 1), np.float32)
                for j in range(t + 1):
                    for h in range(2):
                        lw = et[j][:, h, tl * 128:(tl + 1) * 128]
                        num += lw.T @ VWo[(j + 8 * h) * 128:(j + 8 * h + 1) * 128, :]
                        den += lw.T @ np.ones((128, 1), np.float32)
                outc[t * 128:(t + 1) * 128] = cst(num / den)
        full[b, c::2, :] = outc
    return full
